# revision 12
# baseline (speedup 1.0000x reference)
"""Expert-parallel MoE layer for Trainium2 (8 NeuronCores).

Reference semantics (see reference docstring): top-2 of 16 experts, softmax
gate over the top-2 scores, two-layer MLP per expert (relu between), weighted
combine, plus two aux scalar losses (load variance, importance variance).

Sharding: expert parallelism. Core c holds experts {2c, 2c+1} (W1/b1/W2/b2
slices as per-core inputs). Gating is token-sharded then AllGathered so every
core sees all scores; each core routes + gathers the tokens of its own two
experts from its full copy of x, computes the expert MLPs at a fixed capacity
C per expert, scales rows by gate weight, scatter-adds them into a (B, D)
partial, and a ReduceScatter(add) hands each core the finished 256-token
output shard.  Aux losses are computed (replicated) on-device.

kernel(**inputs) accepts the FULL inputs and returns the FULL outputs
(out, load_loss, importance_loss) matching the reference.
"""

import numpy as np

PT = 128  # SBUF partitions


class MoeProgram:
    """Builds the per-core SPMD Bass program for the MoE layer."""

    def __init__(self, B=2048, D=2048, E=16, M=8, C=384, fp32r=True, stop_after=None):
        import concourse.bass as bass
        import concourse.mybir as mybir
        import concourse.tile as tile
        from concourse import bacc
        from concourse.masks import make_identity, make_upper_triangular

        self.B, self.D, self.E, self.M, self.C = B, D, E, M, C
        do_routing = stop_after not in ('ag',)
        do_p2_les = do_routing and stop_after not in ('p2noles',)
        do_p2_scatter = do_p2_les and stop_after not in ('p2nosc',)
        do_experts = do_routing and stop_after not in ('routing',)
        do_rs = do_experts and stop_after not in ('experts',)
        EL = E // M  # local experts per core
        assert EL == 2, "routing code assumes 2 local experts"
        NB = B // PT          # token tiles
        ND = D // PT          # feature tiles
        SH = B // M           # tokens per core shard
        NBS = SH // PT        # shard token tiles
        NRB = (C + PT - 1) // PT  # capacity row-blocks
        assert C % PT == 0
        OOB = 1 << 20

        dt = mybir.dt
        f32 = dt.float32
        mm_dt = dt.float32r if fp32r else dt.float32

        nc = bacc.Bacc("TRN2", target_bir_lowering=False, debug=False, num_devices=M)
        self.nc = nc

        x_d = nc.dram_tensor("x", [B, D], f32, kind="ExternalInput")
        xs_d = nc.dram_tensor("xs", [SH, D], f32, kind="ExternalInput")
        w1_d = nc.dram_tensor("w1l", [EL, D, D], mm_dt, kind="ExternalInput")
        b1_d = nc.dram_tensor("b1l", [EL, D], f32, kind="ExternalInput")
        w2_d = nc.dram_tensor("w2l", [EL, D, D], mm_dt, kind="ExternalInput")
        b2_d = nc.dram_tensor("b2l", [EL, D], f32, kind="ExternalInput")
        wg_d = nc.dram_tensor("wg", [D, E], f32, kind="ExternalInput")
        bg_d = nc.dram_tensor("bg", [E, 1], f32, kind="ExternalInput")
        lsel_d = nc.dram_tensor("lsel", [EL, PT, E], f32, kind="ExternalInput")
        out_d = nc.dram_tensor("out_shard", [SH, D], f32, kind="ExternalOutput")
        aux_d = nc.dram_tensor("aux", [1, 2], f32, kind="ExternalOutput")

        rg = [list(range(M))]
        AluOp = mybir.AluOpType
        Act = mybir.ActivationFunctionType

        with tile.TileContext(nc) as tc:
            with (
                tc.tile_pool(name="const", bufs=1) as cpool,
                tc.tile_pool(name="dram", bufs=1, space="DRAM") as dram,
            ):
                ident = cpool.tile([PT, PT], f32)
                make_identity(nc, ident[:])
                utri = cpool.tile([PT, PT], f32)
                make_upper_triangular(nc, utri[:], val=1.0, diag=True)
                ones_row = cpool.tile([1, PT], f32)
                nc.vector.memset(ones_row[:], 1.0)
                ones_col = cpool.tile([PT, 1], f32)
                nc.vector.memset(ones_col[:], 1.0)
                iota_f = cpool.tile([PT, 1], f32)
                iota_i = cpool.tile([PT, 1], dt.int32)
                nc.gpsimd.iota(iota_i[:], pattern=[[0, 1]], base=0, channel_multiplier=1)
                nc.vector.tensor_copy(iota_f[:], iota_i[:])
                zeros_big = cpool.tile([PT, D], f32)
                nc.vector.memset(zeros_big[:], 0.0)
                oobfill = cpool.tile([PT, 2], f32)
                nc.vector.memset(oobfill[:, 0:1], float(OOB))
                nc.vector.memset(oobfill[:, 1:2], 0.0)

                wg_sb = cpool.tile([PT, ND * E], f32)
                nc.sync.dma_start(
                    wg_sb[:].rearrange("p (t e) -> p t e", e=E),
                    wg_d[:].rearrange("(t p) e -> p t e", p=PT),
                )
                bg_sb = cpool.tile([E, 1], f32)
                nc.sync.dma_start(bg_sb[:], bg_d[:])
                lsel_sb = cpool.tile([PT, EL * E], f32)
                nc.sync.dma_start(
                    lsel_sb[:].rearrange("p (l e) -> p l e", e=E),
                    lsel_d[:].rearrange("l p e -> p l e"),
                )

                # DRAM scratch
                ag_in = dram.tile([SH, E], f32)
                ag_out = dram.tile([B, E], f32, addr_space="Shared")
                disps = [
                    dram.tile([C, 2], f32, name=f"disp{le}") for le in range(EL)
                ]
                partial = dram.tile([B, D], f32)
                rs_out = dram.tile([SH, D], f32)

                # ---- zero-init partial output and dispatch lists (early) ----
                for t in range(NB):
                    nc.sync.dma_start(partial[t * PT : (t + 1) * PT, :], zeros_big[:])
                for le in range(EL):
                    for rb in range(NRB):
                        nc.sync.dma_start(
                            disps[le][rb * PT : (rb + 1) * PT, :], oobfill[:]
                        )

                # ================= Phase 1: gating (token shard) ==============
                with (
                    tc.tile_pool(name="p1sb", bufs=2) as p1,
                    tc.tile_pool(name="p1ps", bufs=2, space="PSUM") as p1ps,
                    tc.tile_pool(name="p1ps2", bufs=2, space="PSUM") as p1ps2,
                ):
                    for bt in range(NBS):
                        xs_sb = p1.tile([PT, D], f32, tag="xs")
                        nc.sync.dma_start(
                            xs_sb[:], xs_d[bt * PT : (bt + 1) * PT, :]
                        )
                        scT_ps = p1ps.tile([E, PT], f32, space="PSUM", tag="scT")
                        for dtile in range(ND):
                            xT_ps = p1ps2.tile([PT, PT], f32, space="PSUM", tag="xT")
                            nc.tensor.transpose(
                                xT_ps[:],
                                xs_sb[:, dtile * PT : (dtile + 1) * PT],
                                ident[:],
                            )
                            xT_sb = p1.tile([PT, PT], f32, tag="xTsb")
                            nc.vector.tensor_copy(xT_sb[:], xT_ps[:])
                            nc.tensor.matmul(
                                scT_ps[:],
                                wg_sb[:, dtile * E : (dtile + 1) * E],
                                xT_sb[:],
                                start=(dtile == 0),
                                stop=(dtile == ND - 1),
                            )
                        scT_sb = p1.tile([E, PT], f32, tag="scTsb")
                        nc.scalar.activation(
                            scT_sb[:], scT_ps[:], Act.Identity, bias=bg_sb[:, 0:1]
                        )
                        sc_ps = p1ps2.tile([PT, E], f32, space="PSUM", tag="scps")
                        nc.tensor.transpose(
                            sc_ps[:], scT_sb[:], ident[0:E, 0:E]
                        )
                        sc_sb = p1.tile([PT, E], f32, tag="scsb")
                        nc.vector.tensor_copy(sc_sb[:], sc_ps[:])
                        nc.sync.dma_start(
                            ag_in[bt * PT : (bt + 1) * PT, :], sc_sb[:]
                        )
                    nc.gpsimd.collective_compute(
                        "AllGather",
                        AluOp.bypass,
                        replica_groups=rg,
                        ins=[ag_in.opt()],
                        outs=[ag_out.opt()],
                    )

                # ================= Phase 2: routing (replicated) ==============
                run_p2 = do_routing
                loads_sb = cpool.tile([1, E], f32)
                imp_sb = cpool.tile([1, E], f32)
                nc.vector.memset(loads_sb[:], 1.0)
                nc.vector.memset(imp_sb[:], 1.0)
                if run_p2:
                  with (
                    tc.tile_pool(name="p2sb", bufs=3) as p2,
                    tc.tile_pool(name="p2cum", bufs=2, space="PSUM") as p2cum,
                    tc.tile_pool(name="p2carry", bufs=2, space="PSUM") as p2carry,
                    tc.tile_pool(name="p2imp", bufs=1, space="PSUM") as p2imp,
                ):
                    scores_all = cpool.tile([PT, NB * E], f32)
                    nc.sync.dma_start(
                        scores_all[:].rearrange("p (t e) -> p t e", e=E),
                        ag_out[:].rearrange("(t p) e -> p t e", p=PT),
                    )
                    imp_ps = p2imp.tile([1, E], f32, space="PSUM")
                    carry_sb = cpool.tile([1, E], f32)
                    nc.vector.memset(carry_sb[:], 0.0)
                    for t in range(NB):
                        sc = scores_all[:, t * E : (t + 1) * E]
                        mx = p2.tile([PT, 8], f32, tag="mx")
                        nc.vector.max(mx[:], sc)
                        s0 = mx[:, 0:1]
                        s1 = mx[:, 1:2]
                        eq0 = p2.tile([PT, E], f32, tag="eq0")
                        nc.vector.tensor_tensor(
                            out=eq0[:], in0=sc, in1=s0.to_broadcast([PT, E]),
                            op=AluOp.is_equal,
                        )
                        eq1 = p2.tile([PT, E], f32, tag="eq1")
                        nc.vector.tensor_tensor(
                            out=eq1[:], in0=sc, in1=s1.to_broadcast([PT, E]),
                            op=AluOp.is_equal,
                        )
                        mask = p2.tile([PT, E], f32, tag="mask")
                        nc.vector.tensor_add(mask[:], eq0[:], eq1[:])

                        # gate weights over top-2: w0 = sigmoid(s0-s1), w1 = 1-w0
                        diff = p2.tile([PT, 1], f32, tag="diff")
                        nc.vector.tensor_sub(diff[:], s0, s1)
                        w0 = p2.tile([PT, 1], f32, tag="w0")
                        nc.scalar.activation(w0[:], diff[:], Act.Sigmoid)
                        w1 = p2.tile([PT, 1], f32, tag="w1")
                        nc.vector.tensor_scalar(
                            out=w1[:], in0=w0[:], scalar1=-1.0, scalar2=1.0,
                            op0=AluOp.mult, op1=AluOp.add,
                        )
                        wcol = p2.tile([PT, E], f32, tag="wcol")
                        nc.vector.tensor_scalar_mul(wcol[:], eq0[:], w0[:])
                        nc.vector.scalar_tensor_tensor(
                            out=wcol[:], in0=eq1[:], scalar=w1[:], in1=wcol[:],
                            op0=AluOp.mult, op1=AluOp.add,
                        )

                        # importance: softmax over all E, accumulated over tokens
                        neg_s0 = p2.tile([PT, 1], f32, tag="negs0")
                        nc.vector.tensor_scalar_mul(neg_s0[:], s0, -1.0)
                        ex = p2.tile([PT, E], f32, tag="ex")
                        nc.scalar.activation(ex[:], sc, Act.Exp, bias=neg_s0[:, 0:1])
                        sume = p2.tile([PT, 1], f32, tag="sume")
                        nc.vector.tensor_reduce(
                            out=sume[:], in_=ex[:], axis=mybir.AxisListType.X,
                            op=AluOp.add,
                        )
                        rec = p2.tile([PT, 1], f32, tag="rec")
                        nc.vector.reciprocal(rec[:], sume[:])
                        psm = p2.tile([PT, E], f32, tag="psm")
                        nc.vector.tensor_scalar_mul(psm[:], ex[:], rec[:])
                        nc.tensor.matmul(
                            imp_ps[:], ones_col[:], psm[:],
                            start=(t == 0), stop=(t == NB - 1),
                        )

                        # running cumsum: in-tile inclusive prefix + carry row
                        cum_ps = p2cum.tile([PT, E], f32, space="PSUM", tag="cum")
                        nc.tensor.matmul(
                            cum_ps[:], utri[:], mask[:], start=True, stop=False,
                        )
                        nc.tensor.matmul(
                            cum_ps[:], ones_row[:], carry_sb[:],
                            start=False, stop=True,
                        )
                        slot_inc = p2.tile([PT, E], f32, tag="slotinc")
                        nc.vector.tensor_copy(slot_inc[:], cum_ps[:])
                        # carry_next = row 127 of slot_inc, via indicator matmul
                        carry_ps = p2carry.tile([1, E], f32, space="PSUM", tag="carry")
                        nc.tensor.matmul(
                            carry_ps[:], ident[:, PT - 1 : PT], slot_inc[:],
                            start=True, stop=True,
                        )
                        nc.vector.tensor_copy(carry_sb[:], carry_ps[:])
                        slot_exc = p2.tile([PT, E], f32, tag="slotexc")
                        nc.vector.tensor_sub(slot_exc[:], slot_inc[:], mask[:])

                        tokid = p2.tile([PT, 1], f32, tag="tokid")
                        nc.vector.tensor_scalar_add(tokid[:], iota_f[:], float(t * PT))

                        for le in range(EL if do_p2_les else 0):
                            lsel = lsel_sb[:, le * E : (le + 1) * E]
                            scr = p2.tile([PT, E], f32, tag="scr")
                            mask_l = p2.tile([PT, 1], f32, tag="mask_l")
                            nc.vector.tensor_tensor(
                                out=scr[:], in0=mask[:], in1=lsel, op=AluOp.mult)
                            nc.vector.tensor_reduce(
                                out=mask_l[:], in_=scr[:],
                                axis=mybir.AxisListType.X, op=AluOp.add)
                            slot_l = p2.tile([PT, 1], f32, tag="slot_l")
                            nc.vector.tensor_tensor(
                                out=scr[:], in0=slot_exc[:], in1=lsel, op=AluOp.mult)
                            nc.vector.tensor_reduce(
                                out=slot_l[:], in_=scr[:],
                                axis=mybir.AxisListType.X, op=AluOp.add)
                            w_l = p2.tile([PT, 1], f32, tag="w_l")
                            nc.vector.tensor_tensor(
                                out=scr[:], in0=wcol[:], in1=lsel, op=AluOp.mult)
                            nc.vector.tensor_reduce(
                                out=w_l[:], in_=scr[:],
                                axis=mybir.AxisListType.X, op=AluOp.add)
                            # dest = slot_l if routed else OOB
                            dest_f = p2.tile([PT, 1], f32, tag="dest_f")
                            nc.vector.tensor_tensor(
                                out=dest_f[:], in0=slot_l[:], in1=mask_l[:],
                                op=AluOp.mult,
                            )
                            oobt = p2.tile([PT, 1], f32, tag="oobt")
                            nc.vector.tensor_scalar(
                                out=oobt[:], in0=mask_l[:], scalar1=float(-OOB),
                                scalar2=float(OOB), op0=AluOp.mult, op1=AluOp.add,
                            )
                            nc.vector.tensor_add(dest_f[:], dest_f[:], oobt[:])
                            dest_i = p2.tile([PT, 1], dt.int32, tag="dest_i")
                            nc.vector.tensor_copy(dest_i[:], dest_f[:])
                            val = p2.tile([PT, 2], f32, tag="val")
                            nc.vector.tensor_copy(val[:, 0:1], tokid[:])
                            nc.vector.tensor_copy(val[:, 1:2], w_l[:])
                            if do_p2_scatter:
                                nc.gpsimd.indirect_dma_start(
                                    out=disps[le][:],
                                    out_offset=bass_IndirectOffsetOnAxis(dest_i[:, 0:1], 0),
                                    in_=val[:],
                                    in_offset=None,
                                    bounds_check=C - 1,
                                    oob_is_err=False,
                                )
                    nc.vector.tensor_copy(loads_sb[:], carry_sb[:])
                    nc.vector.tensor_copy(imp_sb[:], imp_ps[:])

                # ================= Phase 3: expert MLPs =======================
                if do_experts:
                  with (
                    tc.tile_pool(name="p3w", bufs=2) as p3w,
                    tc.tile_pool(name="p3xg", bufs=2) as p3xg,
                    tc.tile_pool(name="p3big", bufs=1) as p3big,
                    tc.tile_pool(name="p3y", bufs=2) as p3y,
                    tc.tile_pool(name="p3sm", bufs=2) as p3sm,
                    tc.tile_pool(name="p3ps", bufs=2, space="PSUM") as p3ps,
                    tc.tile_pool(name="p3pst", bufs=2, space="PSUM") as p3pst,
                ):
                    for le in range(EL):
                        bias1 = p3sm.tile([PT, ND], f32, tag="bias1")
                        nc.sync.dma_start(
                            bias1[:].rearrange("p (o t) -> p o t", o=1),
                            b1_d[le : le + 1, :].rearrange("o (t p) -> p o t", p=PT),
                        )
                        bias2 = p3sm.tile([PT, ND], f32, tag="bias2")
                        nc.sync.dma_start(
                            bias2[:].rearrange("p (o t) -> p o t", o=1),
                            b2_d[le : le + 1, :].rearrange("o (t p) -> p o t", p=PT),
                        )
                        toks = []
                        wcols = []
                        for rb in range(NRB):
                            dsp = p3sm.tile([PT, 2], f32, tag="dsp")
                            nc.sync.dma_start(
                                dsp[:], disps[le][rb * PT : (rb + 1) * PT, :]
                            )
                            tok_i = p3sm.tile([PT, 1], dt.int32, tag="tok_i", bufs=NRB)
                            nc.vector.tensor_copy(tok_i[:], dsp[:, 0:1])
                            wc = p3sm.tile([PT, 1], f32, tag="wc", bufs=NRB)
                            nc.vector.tensor_copy(wc[:], dsp[:, 1:2])
                            toks.append(tok_i)
                            wcols.append(wc)

                        # gather + transpose tokens -> xgT [d_part, ND*C]
                        xgT = p3big.tile([PT, ND * C], mm_dt, tag="xgT")
                        for rb in range(NRB):
                            xg = p3xg.tile([PT, D], f32, tag="xg")
                            nc.gpsimd.indirect_dma_start(
                                out=xg[:],
                                out_offset=None,
                                in_=x_d[:],
                                in_offset=bass_IndirectOffsetOnAxis(toks[rb][:, 0:1], 0),
                                bounds_check=B - 1,
                                oob_is_err=False,
                            )
                            for dtile in range(ND):
                                tp = p3pst.tile([PT, PT], f32, space="PSUM", tag="tp")
                                nc.tensor.transpose(
                                    tp[:], xg[:, dtile * PT : (dtile + 1) * PT], ident[:]
                                )
                                nc.vector.tensor_copy(
                                    xgT[:, dtile * C + rb * PT : dtile * C + (rb + 1) * PT],
                                    tp[:],
                                )

                        # layer 1: hT[ot] = relu(W1^T xg^T + b1)
                        hT = p3big.tile([PT, ND * C], mm_dt, tag="hT")
                        for ot in range(ND):
                            w1cs = p3w.tile([PT, ND * PT], mm_dt, tag="wcs")
                            nc.sync.dma_start(
                                w1cs[:].rearrange("p (t m) -> p t m", m=PT),
                                w1_d[le : le + 1, :, ot * PT : (ot + 1) * PT].rearrange(
                                    "o (t p) m -> p (o t) m", p=PT
                                ),
                            )
                            h_ps = p3ps.tile([PT, C], f32, space="PSUM", tag="h")
                            for dtile in range(ND):
                                nc.tensor.matmul(
                                    h_ps[:],
                                    w1cs[:, dtile * PT : (dtile + 1) * PT],
                                    xgT[:, dtile * C : (dtile + 1) * C],
                                    start=(dtile == 0),
                                    stop=(dtile == ND - 1),
                                )
                            nc.scalar.activation(
                                hT[:, ot * C : (ot + 1) * C], h_ps[:], Act.Relu,
                                bias=bias1[:, ot : ot + 1],
                            )

                        # layer 2 + transpose back + scale + scatter-add
                        y_rbs = [
                            p3y.tile([PT, D], f32, tag=f"y{rb}", bufs=1, name=f"y_rb{rb}")
                            for rb in range(NRB)
                        ]
                        for ot in range(ND):
                            w2cs = p3w.tile([PT, ND * PT], mm_dt, tag="wcs")
                            nc.sync.dma_start(
                                w2cs[:].rearrange("p (t m) -> p t m", m=PT),
                                w2_d[le : le + 1, :, ot * PT : (ot + 1) * PT].rearrange(
                                    "o (t p) m -> p (o t) m", p=PT
                                ),
                            )
                            y_ps = p3ps.tile([PT, C], f32, space="PSUM", tag="y")
                            for dtile in range(ND):
                                nc.tensor.matmul(
                                    y_ps[:],
                                    w2cs[:, dtile * PT : (dtile + 1) * PT],
                                    hT[:, dtile * C : (dtile + 1) * C],
                                    start=(dtile == 0),
                                    stop=(dtile == ND - 1),
                                )
                            yT = p3sm.tile([PT, C], f32, tag="yT")
                            nc.scalar.activation(
                                yT[:], y_ps[:], Act.Identity,
                                bias=bias2[:, ot : ot + 1],
                            )
                            for rb in range(NRB):
                                tp2 = p3pst.tile([PT, PT], f32, space="PSUM", tag="tp2")
                                nc.tensor.transpose(
                                    tp2[:], yT[:, rb * PT : (rb + 1) * PT], ident[:]
                                )
                                nc.vector.tensor_copy(
                                    y_rbs[rb][:, ot * PT : (ot + 1) * PT], tp2[:]
                                )
                        for rb in range(NRB):
                            nc.vector.tensor_scalar_mul(
                                y_rbs[rb][:], y_rbs[rb][:], wcols[rb][:, 0:1]
                            )
                            nc.gpsimd.indirect_dma_start(
                                out=partial[:],
                                out_offset=bass_IndirectOffsetOnAxis(toks[rb][:, 0:1], 0),
                                in_=y_rbs[rb][:],
                                in_offset=None,
                                bounds_check=B - 1,
                                oob_is_err=False,
                                compute_op=AluOp.add,
                            )

                # ================= Phase 4: ReduceScatter =====================
                if do_rs:
                  nc.gpsimd.collective_compute(
                      "ReduceScatter",
                      AluOp.add,
                      replica_groups=rg,
                      ins=[partial.opt()],
                      outs=[rs_out.opt()],
                  )
                  with tc.tile_pool(name="p4", bufs=2) as p4:
                      for bt in range(NBS):
                        o_sb = p4.tile([PT, D], f32, tag="o")
                        nc.sync.dma_start(o_sb[:], rs_out[bt * PT : (bt + 1) * PT, :])
                        nc.sync.dma_start(out_d[bt * PT : (bt + 1) * PT, :], o_sb[:])

                # ================= Phase 5: aux losses ========================
                with tc.tile_pool(name="p5", bufs=1) as p5:
                    aux_sb = p5.tile([1, 2], f32)
                    scratchE = p5.tile([1, E], f32)
                    mean = p5.tile([1, 1], f32)
                    var = p5.tile([1, 1], f32)
                    # load_loss = var(loads, ddof=1) / B
                    nc.vector.tensor_reduce(
                        out=mean[:], in_=loads_sb[:], axis=mybir.AxisListType.X,
                        op=AluOp.add,
                    )
                    nc.vector.tensor_scalar_mul(mean[:], mean[:], 1.0 / E)
                    nc.vector.tensor_scalar_sub(scratchE[:], loads_sb[:], mean[:, 0:1])
                    nc.vector.tensor_tensor(
                        out=scratchE[:], in0=scratchE[:], in1=scratchE[:],
                        op=AluOp.mult,
                    )
                    nc.vector.tensor_reduce(
                        out=var[:], in_=scratchE[:], axis=mybir.AxisListType.X,
                        op=AluOp.add,
                    )
                    nc.vector.tensor_scalar_mul(
                        aux_sb[:, 0:1], var[:], 1.0 / ((E - 1) * B)
                    )
                    # importance_loss = var(imp, ddof=1) / (mean(imp) + 1e-8)
                    nc.vector.tensor_reduce(
                        out=mean[:], in_=imp_sb[:], axis=mybir.AxisListType.X,
                        op=AluOp.add,
                    )
                    nc.vector.tensor_scalar_mul(mean[:], mean[:], 1.0 / E)
                    nc.vector.tensor_scalar_sub(scratchE[:], imp_sb[:], mean[:, 0:1])
                    nc.vector.tensor_tensor(
                        out=scratchE[:], in0=scratchE[:], in1=scratchE[:],
                        op=AluOp.mult,
                    )
                    nc.vector.tensor_reduce(
                        out=var[:], in_=scratchE[:], axis=mybir.AxisListType.X,
                        op=AluOp.add,
                    )
                    nc.vector.tensor_scalar_mul(var[:], var[:], 1.0 / (E - 1))
                    denom = p5.tile([1, 1], f32)
                    nc.vector.tensor_scalar_add(denom[:], mean[:], 1e-8)
                    rden = p5.tile([1, 1], f32)
                    nc.vector.reciprocal(rden[:], denom[:])
                    nc.vector.tensor_tensor(
                        out=aux_sb[:, 1:2], in0=var[:], in1=rden[:], op=AluOp.mult,
                    )
                    nc.sync.dma_start(aux_d[:], aux_sb[:])

        nc.compile()

    def make_in_maps(self, x, W1, b1, W2, b2, Wg, bg):
        B, D, E, M = self.B, self.D, self.E, self.M
        EL = E // M
        SH = B // M
        in_maps = []
        for c in range(M):
            lsel = np.zeros((EL, PT, E), np.float32)
            for le in range(EL):
                lsel[le, :, EL * c + le] = 1.0
            in_maps.append(
                {
                    "x": np.ascontiguousarray(x),
                    "xs": np.ascontiguousarray(x[SH * c : SH * (c + 1)]),
                    "w1l": np.ascontiguousarray(W1[EL * c : EL * (c + 1)]),
                    "b1l": np.ascontiguousarray(b1[EL * c : EL * (c + 1)]),
                    "w2l": np.ascontiguousarray(W2[EL * c : EL * (c + 1)]),
                    "b2l": np.ascontiguousarray(b2[EL * c : EL * (c + 1)]),
                    "wg": np.ascontiguousarray(Wg),
                    "bg": np.ascontiguousarray(bg.reshape(E, 1)),
                    "lsel": lsel,
                }
            )
        return in_maps

    def assemble(self, results):
        out = np.concatenate([r["out_shard"] for r in results], axis=0)
        aux = results[0]["aux"]
        return out, np.float32(aux[0, 0]), np.float32(aux[0, 1])


def bass_IndirectOffsetOnAxis(ap, axis):
    import concourse.bass as bass

    return bass.IndirectOffsetOnAxis(ap=ap, axis=axis)


_PROGRAM_CACHE = {}


def kernel(x, W1, b1, W2, b2, Wg, bg):
    from concourse import bass_utils

    x = np.asarray(x, np.float32)
    W1 = np.asarray(W1, np.float32)
    b1 = np.asarray(b1, np.float32)
    W2 = np.asarray(W2, np.float32)
    b2 = np.asarray(b2, np.float32)
    Wg = np.asarray(Wg, np.float32)
    bg = np.asarray(bg, np.float32)

    B, D = x.shape
    E = W1.shape[0]
    M = 8
    key = (B, D, E, M)
    if key not in _PROGRAM_CACHE:
        _PROGRAM_CACHE[key] = MoeProgram(B=B, D=D, E=E, M=M, C=384, fp32r=True)
    prog = _PROGRAM_CACHE[key]
    in_maps = prog.make_in_maps(x, W1, b1, W2, b2, Wg, bg)
    res = bass_utils.run_bass_kernel_spmd(
        prog.nc, in_maps, core_ids=list(range(M))
    )
    return prog.assemble(res.results)
